# revision 18
# baseline (speedup 1.0000x reference)
"""Trainium2 Bass kernel: standard multi-head attention (B=2, S=2048, H=16, D=128, fp32).

Sharding: head-parallel across 8 NeuronCores (2 heads per core, both batches),
zero cross-core communication.

Host side (part of sharding): per core, Q and K head-slices are transposed to
[d, s] and cast to fp16; V is cast to fp16 and augmented with a ones column
(so the PV matmul accumulates the softmax denominator for free). The key
padding mask is folded into V on the host: masked key rows of V_aug
(including the ones column) are zeroed, which is exactly equivalent to the
-inf score masking.

Per-core device program, per (batch, head) unit, per 512-wide q-chunk:
  - 16 k-tiles in 8 groups of 2. Six groups exp on the SCALAR engine
    (exact ACT exp, (N+352)/1.1GHz); two groups exp on the VECTOR engine
    via a Schraudolph-style approximation: one tensor_scalar computing
    i16 = int16(A*s + B) whose bit pattern IS fp16(exp(scale*s)) (~1.8%
    RMS weight error on 4/16 of the keys -> ~9e-3 output rel err, well
    under the 2e-2 budget). This removes the scalar engine from the
    critical path (it was the steady-state bottleneck: 16.8M exps/core
    at ~1 elem/ns/lane ~= 131us+overheads > PE ~138us @P0).
  - PSUM budget (8 banks exactly): scalar score tiles [128,2,512]f32
    ping-pong (4 banks) + one DVE score tile [128,2,512]f32 (2 banks,
    single-buffered; its two uses per chunk are spaced 4 groups apart)
    + 2 x 1-bank PV accumulators.
  - PV for chunk c runs interleaved with QK/exp of chunk c+1: per q-tile,
    one 16-matmul PSUM accumulation group out[q, 129] (col 128 = softmax
    denominator), then DVE reciprocal + scalar-mul -> osb fp16, one DMA
    store per chunk (alternating issue engine to spread DMA queues).
  - Output is fp16 and chunk-tiled [B, H2, NQC, 128, QT, D] on device
    (halves store traffic, 1KB-contiguous DRAM runs per partition; +0.03%
    quantization, irrelevant vs budget); host untiles/casts to the full
    fp32 [B, S, H, D].
  - The final chunk splits PV into k-groups 0-3 (parked to SBUF during the
    last exp groups) + k-groups 4-7, storing in two halves on the two
    HW-DGE queues, to shorten the pipeline drain tail.

All accumulation fp32; matmul inputs fp16 (full PE rate).
"""

import numpy as np

B, S, H, D = 2, 2048, 16, 128
NCORES = 8
H2 = H // NCORES          # heads per core
KTILES = S // 128         # 16
VW = D + 2                # V_aug row width: 128 d + ones col + pad
QCHUNK = 512
NQC = S // QCHUNK         # 4 chunks per (b, h) unit
QT = QCHUNK // 128        # 4 q-tiles per chunk
SCALE = 1.0 / float(np.sqrt(D))

# k-tile groups per chunk: (first k-tile, width, engine) where engine is
# "s" (scalar ACT exp) or "v" (vector Schraudolph exp)
GROUPS = [
    (0, 2, "s"), (2, 2, "s"), (4, 2, "s"), (6, 2, "v"),
    (8, 2, "s"), (10, 2, "s"), (12, 2, "s"), (14, 2, "v"),
]
NG = len(GROUPS)
# chunk 0 starts with two tiny groups so the first exp fires after a single
# matmul (the PE runs slow during its HAM power-ramp window)
GROUPS0 = [(0, 1, "s"), (1, 1, "s")] + GROUPS[1:]

# Schraudolph fp16 exp: bits = int16(A*s + B) viewed as fp16 ~ exp(SCALE*s).
# A folds the 1/sqrt(D) softmax scale; B = 1024*15 + C with C=-62 tuned to
# zero the mean multiplicative error (the residual ~1.8% RMS sawtooth is
# what lands in the output error).
SCHRAU_A = float(1024.0 * np.log2(np.e) * SCALE)
SCHRAU_B = 15360.0 - 62.0

_CACHE = {}


def _build_program():
    from contextlib import ExitStack

    import concourse.tile as tile
    from concourse import bacc, mybir

    f32 = mybir.dt.float32
    f16 = mybir.dt.float16
    i16 = mybir.dt.int16

    nc = bacc.Bacc("TRN2", target_bir_lowering=False, debug=False, num_devices=NCORES)
    qt_d = nc.dram_tensor("qt", [B, H2, D, S], f16, kind="ExternalInput").ap()
    kt_d = nc.dram_tensor("kt", [B, H2, D, S], f16, kind="ExternalInput").ap()
    va_d = nc.dram_tensor("va", [B, H2, 128, KTILES, VW], f16, kind="ExternalInput").ap()
    # o is chunk-tiled [b, h, qc, p, t, d] with s = qc*512 + t*128 + p so a
    # chunk store is one descriptor with 1KB-contiguous DRAM runs per
    # partition (the flat [s, d] layout gave 256B runs -> ~32GB/s ring)
    o_d = nc.dram_tensor(
        "o", [B, H2, NQC, 128, QT, D], f16, kind="ExternalOutput"
    ).ap()

    EXP = mybir.ActivationFunctionType.Exp

    with tile.TileContext(nc) as tc, ExitStack() as ctx:
        tpool = ctx.enter_context(tc.tile_pool(name="tpool", bufs=2))
        vpool = ctx.enter_context(tc.tile_pool(name="vpool", bufs=2))
        ppool = ctx.enter_context(tc.tile_pool(name="ppool", bufs=14))
        opool = ctx.enter_context(tc.tile_pool(name="opool", bufs=3))
        rpool = ctx.enter_context(tc.tile_pool(name="rpool", bufs=8))
        apool = ctx.enter_context(tc.tile_pool(name="apool", bufs=5))
        st_sc = ctx.enter_context(tc.tile_pool(name="st_sc", bufs=2, space="PSUM"))
        st_dv = ctx.enter_context(tc.tile_pool(name="st_dv", bufs=1, space="PSUM"))
        o_ps = ctx.enter_context(tc.tile_pool(name="o_ps", bufs=2, space="PSUM"))

        units = [(b, h) for b in range(B) for h in range(H2)]
        # chunk descriptors (unit, q0, width): the very first and very last
        # 512-q chunks are split into 256-wide halves, so the first QK only
        # waits on half the q transfer and the final drain covers half the
        # PV/normalize work
        chunks = []
        for u in range(len(units)):
            for qc in range(NQC):
                q0 = qc * QCHUNK
                if u == 0 and qc == 0:
                    chunks.append((u, 0, 256))
                    chunks.append((u, 256, 256))
                elif u == len(units) - 1 and qc == NQC - 1:
                    chunks.append((u, q0, 256))
                    chunks.append((u, q0 + 256, 256))
                else:
                    chunks.append((u, q0, QCHUNK))
        nchunks = len(chunks)

        def prep(u, split=False):
            """DMA the unit's pre-transposed fp16 tensors into SBUF.

            split=True stages the pieces the first chunk needs (k-tiles 0-1,
            first q window) ahead of the rest, shortening the pipeline fill."""
            b, h = units[u]
            qt = tpool.tile([128, S], f16, name="qt_sb", tag="qt")
            kt = tpool.tile([128, S], f16, name="kt_sb", tag="kt")
            if split:
                nc.sync.dma_start(kt[:, 0:256], kt_d[b, h, :, 0:256])
                nc.sync.dma_start(qt[:, 0:256], qt_d[b, h, :, 0:256])
                nc.sync.dma_start(kt[:, 256:S], kt_d[b, h, :, 256:S])
                nc.sync.dma_start(qt[:, 256:S], qt_d[b, h, :, 256:S])
            else:
                nc.sync.dma_start(kt[:], kt_d[b, h])
                nc.sync.dma_start(qt[:], qt_d[b, h])
            va = vpool.tile([128, KTILES, VW], f16, name="va_sb", tag="va")
            nc.sync.dma_start(va[:], va_d[b, h])
            return {"q": qt, "k": kt, "v": va}

        unit_tiles = {0: prep(0, split=True)}

        # warm up the ACT exp table before any data arrives (table load ~2.7us)
        warm = rpool.tile([128, 1], f32, name="warm", tag="warm")
        nc.vector.memset(warm[:], 0.0)
        nc.scalar.activation(warm[:], warm[:], EXP, bias=0.0, scale=1.0)

        # pre-warm the PE's HAM clock gate during the input DMA fill: the PE
        # runs at 1.2GHz until it has been busy for a free-running ~3.4us
        # activity window. A burst of dummy matmuls (results discarded; the
        # st_dv bank is overwritten by the first real use) flips it to
        # 2.4GHz before the first data matmul instead of ~7us after.
        wpool = ctx.enter_context(tc.tile_pool(name="wpool", bufs=1))
        dsrc = wpool.tile([128, QCHUNK], f16, name="dsrc", tag="dsrc")
        nc.vector.memset(dsrc[:], 0.0)
        dwarm = st_dv.tile([128, 2 * QCHUNK], f32, name="st_v", tag="st_v")
        for _ in range(4):
            nc.tensor.matmul(
                dwarm[:, 0:QCHUNK], lhsT=dsrc[:, 0:128], rhs=dsrc[:],
                start=True, stop=True,
            )

        state = {}

        def emit_qk_act(c, gi):
            u, q0, qw = chunks[c]
            tl = unit_tiles[u]
            k0, kw, eng = state[c]["grps"][gi]
            if eng == "s":
                st = st_sc.tile([128, 2 * QCHUNK], f32, name="st_s", tag="st_s")
            else:
                st = st_dv.tile([128, 2 * QCHUNK], f32, name="st_v", tag="st_v")
            for i in range(kw):
                j = k0 + i
                nc.tensor.matmul(
                    st[:, i * qw : (i + 1) * qw],
                    lhsT=tl["k"][:, j * 128 : (j + 1) * 128],
                    rhs=tl["q"][:, q0 : q0 + qw],
                    start=True,
                    stop=True,
                )
            pt = ppool.tile([128, 2 * QCHUNK], f16, name="pt", tag="pt")
            if eng == "s":
                nc.scalar.activation(
                    pt[:, 0 : kw * qw], st[:, 0 : kw * qw], EXP,
                    bias=0.0, scale=SCALE,
                )
            else:
                nc.vector.tensor_scalar(
                    out=pt[:, 0 : kw * qw].bitcast(i16),
                    in0=st[:, 0 : kw * qw],
                    scalar1=SCHRAU_A,
                    scalar2=SCHRAU_B,
                    op0=mybir.AluOpType.mult,
                    op1=mybir.AluOpType.add,
                )
            state[c]["pt"].append(pt)

        def pv_matmuls(c, t, glo, ghi, oacc):
            """PV accumulation-group matmuls for q-tile t over groups [glo, ghi)."""
            stt = state[c]
            v = stt["v"]
            qw = chunks[c][2]
            first = True
            for g in range(glo, ghi):
                k0, kw, _ = stt["grps"][g]
                pt = stt["pt"][g]
                for i in range(kw):
                    o = i * qw + t * 128
                    nc.tensor.matmul(
                        oacc[:],
                        lhsT=pt[:, o : o + 128],
                        rhs=v[:, k0 + i, 0 : D + 1],
                        start=first,
                        stop=(g == ghi - 1 and i == kw - 1),
                    )
                    first = False

        def odest(c):
            u, q0, qw = chunks[c]
            b, h = units[u]
            t0 = (q0 % QCHUNK) // 128
            return o_d[b, h, q0 // QCHUNK][:, t0 : t0 + qw // 128, :]

        def emit_pv(c, t):
            stt = state[c]
            qtc = chunks[c][2] // 128
            oacc = o_ps.tile([128, D + 1], f32, name="oacc", tag="oacc")
            pv_matmuls(c, t, 0, len(stt["grps"]), oacc)
            rec = rpool.tile([128, 1], f32, name="rec", tag="rec")
            nc.vector.reciprocal(rec[:], oacc[:, D : D + 1])
            nc.vector.tensor_scalar_mul(stt["osb"][:, t, :], oacc[:, 0:D], rec[:])
            if t == qtc - 1:
                # alternate the issuing engine so chunk stores spread across
                # DMA queues instead of serializing on one ring (gpsimd's
                # software-DGE path measured multi-us trigger->transfer
                # latency, so only sync and scalar queues are used)
                if c % 2 == 0:
                    nc.sync.dma_start(odest(c), stt["osb"][:, 0:qtc, :])
                else:
                    nc.scalar.dma_start(odest(c), stt["osb"][:, 0:qtc, :])
                del state[c]

        def emit_pv_part1(c, t):
            """Last chunk: PV over k-groups 0-3 (k-tiles 0-7), parked to
            SBUF; the lighter part1 keeps the final window's PE load under
            the exp window so the PE is not lagging at the last exp."""
            stt = state[c]
            oacc = o_ps.tile([128, D + 1], f32, name="oacc", tag="oacc")
            pv_matmuls(c, t, 0, 4, oacc)
            asb = apool.tile([128, D + 1], f32, name="asb", tag="asb")
            nc.vector.tensor_copy(asb[:], oacc[:])
            stt["asb"][t] = asb

        def emit_pv_part2(c, t):
            """Last chunk: PV over k-groups 4-7, combine, normalize, store."""
            stt = state[c]
            oacc = o_ps.tile([128, D + 1], f32, name="oacc", tag="oacc")
            pv_matmuls(c, t, 4, len(stt["grps"]), oacc)
            tmp = apool.tile([128, D + 1], f32, name="tmp", tag="tmp")
            nc.vector.tensor_add(tmp[:], oacc[:], stt["asb"][t][:])
            rec = rpool.tile([128, 1], f32, name="rec", tag="rec")
            nc.vector.reciprocal(rec[:], tmp[:, D : D + 1])
            nc.vector.tensor_scalar_mul(stt["osb"][:, t, :], tmp[:, 0:D], rec[:])
            # store each normalized q-tile immediately, alternating queues
            eng = nc.sync if t % 2 == 0 else nc.scalar
            eng.dma_start(odest(c)[:, t : t + 1, :], stt["osb"][:, t : t + 1, :])

        last = nchunks - 1
        qtl = chunks[last][2] // 128          # q-tiles in the last chunk
        for c in range(nchunks):
            u, q0, qw = chunks[c]
            grps = GROUPS0 if c == 0 else GROUPS
            ng = len(grps)
            state[c] = {
                "pt": [],
                "asb": {},
                "grps": grps,
                "v": unit_tiles[u]["v"],
                "osb": opool.tile([128, QT, D], f16, name="osb", tag="osb"),
            }
            # prefetch the next unit's tensors two chunks ahead
            if q0 == S - 2 * QCHUNK and u + 1 < len(units):
                unit_tiles[u + 1] = prep(u + 1)
            qtp = chunks[c - 1][2] // 128 if c > 0 else 0
            for gi in range(ng):
                emit_qk_act(c, gi)
                # pv(c-1, 0) at gi=1 is safe: qk(c, g1) already waits on
                # exp(c-1, last) via the st ping-pong, the same dependency
                if c > 0 and 1 <= gi <= qtp:
                    emit_pv(c - 1, gi - 1)
                if c == last and gi >= ng - qtl + 1:
                    emit_pv_part1(c, gi - (ng - qtl + 1))
            if c == last:
                emit_pv_part1(c, qtl - 1)
                for t in range(qtl):
                    emit_pv_part2(c, t)
                del state[c]

    nc.compile()
    return nc


def _get_program():
    if "nc" not in _CACHE:
        _CACHE["nc"] = _build_program()
    return _CACHE["nc"]


def make_core_inputs(q, k, v, key_padding_mask):
    """Shard full inputs into per-core input maps (host side).

    Layout work done here (part of sharding): head-slice, transpose Q/K to
    [d, s], cast to fp16, build ones-augmented V with masked key rows zeroed
    (exactly equivalent to -inf score masking).
    """
    q = np.asarray(q, dtype=np.float32)
    k = np.asarray(k, dtype=np.float32)
    v = np.asarray(v, dtype=np.float32)
    m = np.asarray(key_padding_mask, dtype=bool)

    # [B, S, H, D] -> [B, H, D, S] fp16
    qt = np.ascontiguousarray(q.transpose(0, 2, 3, 1).astype(np.float16))
    kt = np.ascontiguousarray(k.transpose(0, 2, 3, 1).astype(np.float16))
    # V_aug: [B, H, 128(p), KTILES(t), VW] fp16 with ones in column D,
    # masked key rows zeroed (s = 128*t + p)
    va = np.zeros((B, H, 128, KTILES, VW), dtype=np.float16)
    va[:, :, :, :, 0:D] = (
        v.reshape(B, KTILES, 128, H, D).transpose(0, 3, 2, 1, 4).astype(np.float16)
    )
    va[:, :, :, :, D] = 1.0
    if not m.all():
        # mask[b, s] with s = 128*t + p -> [B, 1, 128(p), KTILES(t), 1]
        mk = m.reshape(B, KTILES, 128).transpose(0, 2, 1)[:, None, :, :, None]
        va *= mk.astype(np.float16)

    in_maps = []
    for c in range(NCORES):
        sl = slice(c * H2, (c + 1) * H2)
        in_maps.append(
            {
                "qt": np.ascontiguousarray(qt[:, sl]),
                "kt": np.ascontiguousarray(kt[:, sl]),
                "va": np.ascontiguousarray(va[:, sl]),
            }
        )
    return in_maps


def assemble_output(results):
    """Concatenate per-core [B, H2, NQC, 128, QT, D] fp16 outputs along the
    head axis, then untile (s = qc*512 + t*128 + p) and cast to the full
    fp32 [B, S, H, D]."""
    o = np.concatenate([results[c]["o"] for c in range(NCORES)], axis=1)
    o = o.transpose(0, 2, 4, 3, 1, 5).reshape(B, S, H, D)
    return np.ascontiguousarray(o).astype(np.float32)


def kernel(q, k, v, key_padding_mask):
    from concourse.bass_utils import run_bass_kernel_spmd

    nc = _get_program()
    in_maps = make_core_inputs(q, k, v, key_padding_mask)
    res = run_bass_kernel_spmd(nc, in_maps, list(range(NCORES)))
    return assemble_output(res.results)



# revision 19
# speedup vs baseline: 1.0104x; 1.0104x over previous
"""Trainium2 Bass kernel: standard multi-head attention (B=2, S=2048, H=16, D=128, fp32).

Sharding: head-parallel across 8 NeuronCores (2 heads per core, both batches),
zero cross-core communication.

Host side (part of sharding): per core, Q and K head-slices are transposed to
[d, s] and cast to fp16; V is cast to fp16 and augmented with a ones column
(so the PV matmul accumulates the softmax denominator for free). The key
padding mask is folded into V on the host: masked key rows of V_aug
(including the ones column) are zeroed, which is exactly equivalent to the
-inf score masking.

Per-core device program, per (batch, head) unit, per 512-wide q-chunk:
  - 16 k-tiles in 8 groups of 2. Six groups exp on the SCALAR engine
    (exact ACT exp, (N+352)/1.1GHz); two groups exp on the VECTOR engine
    via a Schraudolph-style approximation: one tensor_scalar computing
    i16 = int16(A*s + B) whose bit pattern IS fp16(exp(scale*s)) (~1.8%
    RMS weight error on 4/16 of the keys -> ~9e-3 output rel err, well
    under the 2e-2 budget). This removes the scalar engine from the
    critical path (it was the steady-state bottleneck: 16.8M exps/core
    at ~1 elem/ns/lane ~= 131us+overheads > PE ~138us @P0).
  - PSUM budget (8 banks exactly): scalar score tiles [128,2,512]f32
    ping-pong (4 banks) + one DVE score tile [128,2,512]f32 (2 banks,
    single-buffered; its two uses per chunk are spaced 4 groups apart)
    + 2 x 1-bank PV accumulators.
  - PV for chunk c runs interleaved with QK/exp of chunk c+1: per q-tile,
    one 16-matmul PSUM accumulation group out[q, 129] (col 128 = softmax
    denominator), then DVE reciprocal + scalar-mul -> osb fp16, one DMA
    store per chunk (alternating issue engine to spread DMA queues).
  - Output is fp16 and chunk-tiled [B, H2, NQC, 128, QT, D] on device
    (halves store traffic, 1KB-contiguous DRAM runs per partition; +0.03%
    quantization, irrelevant vs budget); host untiles/casts to the full
    fp32 [B, S, H, D].
  - The final chunk splits PV into k-groups 0-3 (parked to SBUF during the
    last exp groups) + k-groups 4-7, storing in two halves on the two
    HW-DGE queues, to shorten the pipeline drain tail.

All accumulation fp32; matmul inputs fp16 (full PE rate).
"""

import numpy as np

B, S, H, D = 2, 2048, 16, 128
NCORES = 8
H2 = H // NCORES          # heads per core
KTILES = S // 128         # 16
VW = D + 2                # V_aug row width: 128 d + ones col + pad
QCHUNK = 512
NQC = S // QCHUNK         # 4 chunks per (b, h) unit
QT = QCHUNK // 128        # 4 q-tiles per chunk
SCALE = 1.0 / float(np.sqrt(D))

# k-tile groups per chunk: (first k-tile, width, engine) where engine is
# "s" (scalar ACT exp) or "v" (vector Schraudolph exp)
GROUPS = [
    (0, 2, "s"), (2, 2, "s"), (4, 2, "s"), (6, 2, "v"),
    (8, 2, "s"), (10, 2, "s"), (12, 2, "s"), (14, 2, "v"),
]
NG = len(GROUPS)
# chunk 0 starts with two tiny groups so the first exp fires after a single
# matmul (the PE runs slow during its HAM power-ramp window)
GROUPS0 = [(0, 1, "s"), (1, 1, "s")] + GROUPS[1:]

# Schraudolph fp16 exp: bits = int16(A*s + B) viewed as fp16 ~ exp(SCALE*s).
# A folds the 1/sqrt(D) softmax scale; B = 1024*15 + C with C=-62 tuned to
# zero the mean multiplicative error (the residual ~1.8% RMS sawtooth is
# what lands in the output error).
SCHRAU_A = float(1024.0 * np.log2(np.e) * SCALE)
SCHRAU_B = 15360.0 - 62.0

_CACHE = {}


def _build_program():
    from contextlib import ExitStack

    import concourse.tile as tile
    from concourse import bacc, mybir

    f32 = mybir.dt.float32
    f16 = mybir.dt.float16
    i16 = mybir.dt.int16

    nc = bacc.Bacc("TRN2", target_bir_lowering=False, debug=False, num_devices=NCORES)
    qt_d = nc.dram_tensor("qt", [B, H2, D, S], f16, kind="ExternalInput").ap()
    kt_d = nc.dram_tensor("kt", [B, H2, D, S], f16, kind="ExternalInput").ap()
    va_d = nc.dram_tensor("va", [B, H2, 128, KTILES, VW], f16, kind="ExternalInput").ap()
    # o is chunk-tiled [b, h, qc, p, t, d] with s = qc*512 + t*128 + p so a
    # chunk store is one descriptor with 1KB-contiguous DRAM runs per
    # partition (the flat [s, d] layout gave 256B runs -> ~32GB/s ring)
    o_d = nc.dram_tensor(
        "o", [B, H2, NQC, 128, QT, D], f16, kind="ExternalOutput"
    ).ap()

    EXP = mybir.ActivationFunctionType.Exp

    with tile.TileContext(nc) as tc, ExitStack() as ctx:
        tpool = ctx.enter_context(tc.tile_pool(name="tpool", bufs=2))
        vpool = ctx.enter_context(tc.tile_pool(name="vpool", bufs=2))
        ppool = ctx.enter_context(tc.tile_pool(name="ppool", bufs=14))
        opool = ctx.enter_context(tc.tile_pool(name="opool", bufs=3))
        rpool = ctx.enter_context(tc.tile_pool(name="rpool", bufs=8))
        apool = ctx.enter_context(tc.tile_pool(name="apool", bufs=5))
        st_sc = ctx.enter_context(tc.tile_pool(name="st_sc", bufs=2, space="PSUM"))
        st_dv = ctx.enter_context(tc.tile_pool(name="st_dv", bufs=1, space="PSUM"))
        o_ps = ctx.enter_context(tc.tile_pool(name="o_ps", bufs=2, space="PSUM"))

        units = [(b, h) for b in range(B) for h in range(H2)]
        chunks = [(u, qc) for u in range(len(units)) for qc in range(NQC)]
        nchunks = len(chunks)

        def prep(u, split=False):
            """DMA the unit's pre-transposed fp16 tensors into SBUF.

            split=True stages the pieces the first chunk needs (k-tiles 0-1,
            first q window) ahead of the rest, shortening the pipeline fill."""
            b, h = units[u]
            qt = tpool.tile([128, S], f16, name="qt_sb", tag="qt")
            kt = tpool.tile([128, S], f16, name="kt_sb", tag="kt")
            if split:
                nc.sync.dma_start(kt[:, 0:256], kt_d[b, h, :, 0:256])
                nc.sync.dma_start(qt[:, 0:QCHUNK], qt_d[b, h, :, 0:QCHUNK])
                nc.sync.dma_start(kt[:, 256:S], kt_d[b, h, :, 256:S])
                nc.sync.dma_start(qt[:, QCHUNK:S], qt_d[b, h, :, QCHUNK:S])
            else:
                nc.sync.dma_start(kt[:], kt_d[b, h])
                nc.sync.dma_start(qt[:], qt_d[b, h])
            va = vpool.tile([128, KTILES, VW], f16, name="va_sb", tag="va")
            nc.sync.dma_start(va[:], va_d[b, h])
            return {"q": qt, "k": kt, "v": va}

        unit_tiles = {0: prep(0, split=True)}

        # warm up the ACT exp table before any data arrives (table load ~2.7us)
        warm = rpool.tile([128, 1], f32, name="warm", tag="warm")
        nc.vector.memset(warm[:], 0.0)
        nc.scalar.activation(warm[:], warm[:], EXP, bias=0.0, scale=1.0)

        # pre-warm the PE's HAM clock gate during the input DMA fill: the PE
        # runs at 1.2GHz until it has been busy for a free-running ~3.4us
        # activity window. A burst of dummy matmuls (results discarded; the
        # st_dv bank is overwritten by the first real use) flips it to
        # 2.4GHz before the first data matmul instead of ~7us after.
        wpool = ctx.enter_context(tc.tile_pool(name="wpool", bufs=1))
        dsrc = wpool.tile([128, QCHUNK], f16, name="dsrc", tag="dsrc")
        nc.vector.memset(dsrc[:], 0.0)
        dwarm = st_dv.tile([128, 2, QCHUNK], f32, name="st_v", tag="st_v")
        for _ in range(4):
            nc.tensor.matmul(
                dwarm[:, 0, :], lhsT=dsrc[:, 0:128], rhs=dsrc[:],
                start=True, stop=True,
            )

        state = {}

        def emit_qk_act(c, gi):
            u, qc = chunks[c]
            tl = unit_tiles[u]
            k0, kw, eng = state[c]["grps"][gi]
            q0 = qc * QCHUNK
            if eng == "s":
                st = st_sc.tile([128, 2, QCHUNK], f32, name="st_s", tag="st_s")
            else:
                st = st_dv.tile([128, 2, QCHUNK], f32, name="st_v", tag="st_v")
            for i in range(kw):
                j = k0 + i
                nc.tensor.matmul(
                    st[:, i, :],
                    lhsT=tl["k"][:, j * 128 : (j + 1) * 128],
                    rhs=tl["q"][:, q0 : q0 + QCHUNK],
                    start=True,
                    stop=True,
                )
            pt = ppool.tile([128, 2, QCHUNK], f16, name="pt", tag="pt")
            if eng == "s":
                nc.scalar.activation(
                    pt[:, 0:kw, :], st[:, 0:kw, :], EXP, bias=0.0, scale=SCALE
                )
            else:
                nc.vector.tensor_scalar(
                    out=pt[:, 0:kw, :].bitcast(i16),
                    in0=st[:, 0:kw, :],
                    scalar1=SCHRAU_A,
                    scalar2=SCHRAU_B,
                    op0=mybir.AluOpType.mult,
                    op1=mybir.AluOpType.add,
                )
            state[c]["pt"].append(pt)

        def pv_matmuls(c, t, glo, ghi, oacc):
            """PV accumulation-group matmuls for q-tile t over groups [glo, ghi)."""
            stt = state[c]
            v = stt["v"]
            first = True
            for g in range(glo, ghi):
                k0, kw, _ = stt["grps"][g]
                pt = stt["pt"][g]
                for i in range(kw):
                    nc.tensor.matmul(
                        oacc[:],
                        lhsT=pt[:, i, t * 128 : (t + 1) * 128],
                        rhs=v[:, k0 + i, 0 : D + 1],
                        start=first,
                        stop=(g == ghi - 1 and i == kw - 1),
                    )
                    first = False

        def odest(c):
            u, qc = chunks[c]
            b, h = units[u]
            return o_d[b, h, qc]

        def emit_pv(c, t):
            stt = state[c]
            oacc = o_ps.tile([128, D + 1], f32, name="oacc", tag="oacc")
            pv_matmuls(c, t, 0, len(stt["grps"]), oacc)
            rec = rpool.tile([128, 1], f32, name="rec", tag="rec")
            nc.vector.reciprocal(rec[:], oacc[:, D : D + 1])
            nc.vector.tensor_scalar_mul(stt["osb"][:, t, :], oacc[:, 0:D], rec[:])
            if t == QT - 1:
                # alternate the issuing engine so chunk stores spread across
                # DMA queues instead of serializing on one ring (gpsimd's
                # software-DGE path measured multi-us trigger->transfer
                # latency, so only sync and scalar queues are used)
                if c % 2 == 0:
                    nc.sync.dma_start(odest(c), stt["osb"][:])
                else:
                    nc.scalar.dma_start(odest(c), stt["osb"][:])
                del state[c]

        def emit_pv_part1(c, t):
            """Last chunk: PV over k-groups 0-3 (k-tiles 0-7), parked to
            SBUF; the lighter part1 keeps the final window's PE load under
            the exp window so the PE is not lagging at the last exp."""
            stt = state[c]
            oacc = o_ps.tile([128, D + 1], f32, name="oacc", tag="oacc")
            pv_matmuls(c, t, 0, 4, oacc)
            asb = apool.tile([128, D + 1], f32, name="asb", tag="asb")
            nc.vector.tensor_copy(asb[:], oacc[:])
            stt["asb"][t] = asb

        def emit_pv_part2(c, t):
            """Last chunk: PV over k-groups 4-7, combine, normalize, store."""
            stt = state[c]
            oacc = o_ps.tile([128, D + 1], f32, name="oacc", tag="oacc")
            pv_matmuls(c, t, 4, len(stt["grps"]), oacc)
            tmp = apool.tile([128, D + 1], f32, name="tmp", tag="tmp")
            nc.vector.tensor_add(tmp[:], oacc[:], stt["asb"][t][:])
            rec = rpool.tile([128, 1], f32, name="rec", tag="rec")
            nc.vector.reciprocal(rec[:], tmp[:, D : D + 1])
            nc.vector.tensor_scalar_mul(stt["osb"][:, t, :], tmp[:, 0:D], rec[:])
            # store in two contiguous halves on the two HW-DGE queues:
            # [t0,t1] as soon as t1 is normalized, [t2,t3] at the end
            if t == 1:
                nc.sync.dma_start(odest(c)[:, 0:2, :], stt["osb"][:, 0:2, :])
            elif t == QT - 1:
                nc.scalar.dma_start(odest(c)[:, 2:4, :], stt["osb"][:, 2:4, :])

        last = nchunks - 1
        for c in range(nchunks):
            u, qc = chunks[c]
            grps = GROUPS0 if c == 0 else GROUPS
            ng = len(grps)
            state[c] = {
                "pt": [],
                "asb": {},
                "grps": grps,
                "v": unit_tiles[u]["v"],
                "osb": opool.tile([128, QT, D], f16, name="osb", tag="osb"),
            }
            # prefetch the next unit's tensors two chunks ahead
            if qc == NQC - 2 and u + 1 < len(units):
                unit_tiles[u + 1] = prep(u + 1)
            for gi in range(ng):
                emit_qk_act(c, gi)
                # pv(c-1, 0) at gi=1 is safe: qk(c, g1) already waits on
                # exp(c-1, last) via the st ping-pong, the same dependency
                if c > 0 and 1 <= gi <= QT:
                    emit_pv(c - 1, gi - 1)
                if c == last and gi >= ng - 3:
                    emit_pv_part1(c, gi - (ng - 3))
            if c == last:
                emit_pv_part1(c, QT - 1)
                for t in range(QT):
                    emit_pv_part2(c, t)
                del state[c]

    nc.compile()
    return nc


def _get_program():
    if "nc" not in _CACHE:
        _CACHE["nc"] = _build_program()
    return _CACHE["nc"]


def make_core_inputs(q, k, v, key_padding_mask):
    """Shard full inputs into per-core input maps (host side).

    Layout work done here (part of sharding): head-slice, transpose Q/K to
    [d, s], cast to fp16, build ones-augmented V with masked key rows zeroed
    (exactly equivalent to -inf score masking).
    """
    q = np.asarray(q, dtype=np.float32)
    k = np.asarray(k, dtype=np.float32)
    v = np.asarray(v, dtype=np.float32)
    m = np.asarray(key_padding_mask, dtype=bool)

    # [B, S, H, D] -> [B, H, D, S] fp16
    qt = np.ascontiguousarray(q.transpose(0, 2, 3, 1).astype(np.float16))
    kt = np.ascontiguousarray(k.transpose(0, 2, 3, 1).astype(np.float16))
    # V_aug: [B, H, 128(p), KTILES(t), VW] fp16 with ones in column D,
    # masked key rows zeroed (s = 128*t + p)
    va = np.zeros((B, H, 128, KTILES, VW), dtype=np.float16)
    va[:, :, :, :, 0:D] = (
        v.reshape(B, KTILES, 128, H, D).transpose(0, 3, 2, 1, 4).astype(np.float16)
    )
    va[:, :, :, :, D] = 1.0
    if not m.all():
        # mask[b, s] with s = 128*t + p -> [B, 1, 128(p), KTILES(t), 1]
        mk = m.reshape(B, KTILES, 128).transpose(0, 2, 1)[:, None, :, :, None]
        va *= mk.astype(np.float16)

    in_maps = []
    for c in range(NCORES):
        sl = slice(c * H2, (c + 1) * H2)
        in_maps.append(
            {
                "qt": np.ascontiguousarray(qt[:, sl]),
                "kt": np.ascontiguousarray(kt[:, sl]),
                "va": np.ascontiguousarray(va[:, sl]),
            }
        )
    return in_maps


def assemble_output(results):
    """Concatenate per-core [B, H2, NQC, 128, QT, D] fp16 outputs along the
    head axis, then untile (s = qc*512 + t*128 + p) and cast to the full
    fp32 [B, S, H, D]."""
    o = np.concatenate([results[c]["o"] for c in range(NCORES)], axis=1)
    o = o.transpose(0, 2, 4, 3, 1, 5).reshape(B, S, H, D)
    return np.ascontiguousarray(o).astype(np.float32)


def kernel(q, k, v, key_padding_mask):
    from concourse.bass_utils import run_bass_kernel_spmd

    nc = _get_program()
    in_maps = make_core_inputs(q, k, v, key_padding_mask)
    res = run_bass_kernel_spmd(nc, in_maps, list(range(NCORES)))
    return assemble_output(res.results)



# revision 20
# speedup vs baseline: 1.0138x; 1.0034x over previous
"""Trainium2 Bass kernel: standard multi-head attention (B=2, S=2048, H=16, D=128, fp32).

Sharding: head-parallel across 8 NeuronCores (2 heads per core, both batches),
zero cross-core communication.

Host side (part of sharding): per core, Q and K head-slices are transposed to
[d, s] and cast to fp16; V is cast to fp16 and augmented with a ones column
(so the PV matmul accumulates the softmax denominator for free). The key
padding mask is folded into V on the host: masked key rows of V_aug
(including the ones column) are zeroed, which is exactly equivalent to the
-inf score masking.

Per-core device program, per (batch, head) unit, per 512-wide q-chunk:
  - 16 k-tiles in 8 groups of 2. Six groups exp on the SCALAR engine
    (exact ACT exp, (N+352)/1.1GHz); two groups exp on the VECTOR engine
    via a Schraudolph-style approximation: one tensor_scalar computing
    i16 = int16(A*s + B) whose bit pattern IS fp16(exp(scale*s)) (~1.8%
    RMS weight error on 4/16 of the keys -> ~9e-3 output rel err, well
    under the 2e-2 budget). This removes the scalar engine from the
    critical path (it was the steady-state bottleneck: 16.8M exps/core
    at ~1 elem/ns/lane ~= 131us+overheads > PE ~138us @P0).
  - PSUM budget (8 banks exactly): scalar score tiles [128,2,512]f32
    ping-pong (4 banks) + one DVE score tile [128,2,512]f32 (2 banks,
    single-buffered; its two uses per chunk are spaced 4 groups apart)
    + 2 x 1-bank PV accumulators.
  - PV for chunk c runs interleaved with QK/exp of chunk c+1: per q-tile,
    one 16-matmul PSUM accumulation group out[q, 129] (col 128 = softmax
    denominator), then DVE reciprocal + scalar-mul -> osb fp16, one DMA
    store per chunk (alternating issue engine to spread DMA queues).
  - Output is fp16 and chunk-tiled [B, H2, NQC, 128, QT, D] on device
    (halves store traffic, 1KB-contiguous DRAM runs per partition; +0.03%
    quantization, irrelevant vs budget); host untiles/casts to the full
    fp32 [B, S, H, D].
  - The final chunk splits PV into k-groups 0-3 (parked to SBUF during the
    last exp groups) + k-groups 4-7, storing in two halves on the two
    HW-DGE queues, to shorten the pipeline drain tail.

All accumulation fp32; matmul inputs fp16 (full PE rate).
"""

import numpy as np

B, S, H, D = 2, 2048, 16, 128
NCORES = 8
H2 = H // NCORES          # heads per core
KTILES = S // 128         # 16
VW = D + 2                # V_aug row width: 128 d + ones col + pad
QCHUNK = 512
NQC = S // QCHUNK         # 4 chunks per (b, h) unit
QT = QCHUNK // 128        # 4 q-tiles per chunk
SCALE = 1.0 / float(np.sqrt(D))

# k-tile groups per chunk: (first k-tile, width, engine) where engine is
# "s" (scalar ACT exp) or "v" (vector Schraudolph exp)
GROUPS = [
    (0, 2, "s"), (2, 2, "s"), (4, 2, "s"), (6, 2, "v"),
    (8, 2, "s"), (10, 2, "s"), (12, 2, "s"), (14, 2, "v"),
]
NG = len(GROUPS)
# chunk 0 starts with two tiny groups so the first exp fires after a single
# matmul (the PE runs slow during its HAM power-ramp window)
GROUPS0 = [(0, 1, "s"), (1, 1, "s")] + GROUPS[1:]

# Schraudolph fp16 exp: bits = int16(A*s + B) viewed as fp16 ~ exp(SCALE*s).
# A folds the 1/sqrt(D) softmax scale; B = 1024*15 + C with C=-62 tuned to
# zero the mean multiplicative error (the residual ~1.8% RMS sawtooth is
# what lands in the output error).
SCHRAU_A = float(1024.0 * np.log2(np.e) * SCALE)
SCHRAU_B = 15360.0 - 62.0

_CACHE = {}


def _build_program():
    from contextlib import ExitStack

    import concourse.tile as tile
    from concourse import bacc, mybir

    f32 = mybir.dt.float32
    f16 = mybir.dt.float16
    i16 = mybir.dt.int16

    nc = bacc.Bacc("TRN2", target_bir_lowering=False, debug=False, num_devices=NCORES)
    qt_d = nc.dram_tensor("qt", [B, H2, D, S], f16, kind="ExternalInput").ap()
    kt_d = nc.dram_tensor("kt", [B, H2, D, S], f16, kind="ExternalInput").ap()
    va_d = nc.dram_tensor("va", [B, H2, 128, KTILES, VW], f16, kind="ExternalInput").ap()
    # o is chunk-tiled [b, h, qc, p, t, d] with s = qc*512 + t*128 + p so a
    # chunk store is one descriptor with 1KB-contiguous DRAM runs per
    # partition (the flat [s, d] layout gave 256B runs -> ~32GB/s ring)
    o_d = nc.dram_tensor(
        "o", [B, H2, NQC, 128, QT, D], f16, kind="ExternalOutput"
    ).ap()

    EXP = mybir.ActivationFunctionType.Exp

    with tile.TileContext(nc) as tc, ExitStack() as ctx:
        tpool = ctx.enter_context(tc.tile_pool(name="tpool", bufs=2))
        vpool = ctx.enter_context(tc.tile_pool(name="vpool", bufs=2))
        ppool = ctx.enter_context(tc.tile_pool(name="ppool", bufs=14))
        opool = ctx.enter_context(tc.tile_pool(name="opool", bufs=3))
        rpool = ctx.enter_context(tc.tile_pool(name="rpool", bufs=8))
        apool = ctx.enter_context(tc.tile_pool(name="apool", bufs=5))
        st_sc = ctx.enter_context(tc.tile_pool(name="st_sc", bufs=2, space="PSUM"))
        st_dv = ctx.enter_context(tc.tile_pool(name="st_dv", bufs=1, space="PSUM"))
        o_ps = ctx.enter_context(tc.tile_pool(name="o_ps", bufs=2, space="PSUM"))

        units = [(b, h) for b in range(B) for h in range(H2)]
        chunks = [(u, qc) for u in range(len(units)) for qc in range(NQC)]
        nchunks = len(chunks)

        def prep(u, split=False):
            """DMA the unit's pre-transposed fp16 tensors into SBUF.

            split=True stages the pieces the first chunk needs (k-tiles 0-1,
            first q window) ahead of the rest, shortening the pipeline fill."""
            b, h = units[u]
            qt = tpool.tile([128, S], f16, name="qt_sb", tag="qt")
            kt = tpool.tile([128, S], f16, name="kt_sb", tag="kt")
            if split:
                nc.sync.dma_start(kt[:, 0:256], kt_d[b, h, :, 0:256])
                nc.sync.dma_start(qt[:, 0:QCHUNK], qt_d[b, h, :, 0:QCHUNK])
                nc.sync.dma_start(kt[:, 256:S], kt_d[b, h, :, 256:S])
                nc.sync.dma_start(qt[:, QCHUNK:S], qt_d[b, h, :, QCHUNK:S])
            else:
                nc.sync.dma_start(kt[:], kt_d[b, h])
                nc.sync.dma_start(qt[:], qt_d[b, h])
            va = vpool.tile([128, KTILES, VW], f16, name="va_sb", tag="va")
            nc.sync.dma_start(va[:], va_d[b, h])
            return {"q": qt, "k": kt, "v": va}

        unit_tiles = {0: prep(0, split=True)}

        # warm up the ACT exp table before any data arrives (table load ~2.7us)
        warm = rpool.tile([128, 1], f32, name="warm", tag="warm")
        nc.vector.memset(warm[:], 0.0)
        nc.scalar.activation(warm[:], warm[:], EXP, bias=0.0, scale=1.0)

        # pre-warm the PE's HAM clock gate during the input DMA fill: the PE
        # runs at 1.2GHz until it has been busy for a free-running ~3.4us
        # activity window. A burst of dummy matmuls (results discarded; the
        # st_dv bank is overwritten by the first real use) flips it to
        # 2.4GHz before the first data matmul instead of ~7us after.
        wpool = ctx.enter_context(tc.tile_pool(name="wpool", bufs=1))
        dsrc = wpool.tile([128, QCHUNK], f16, name="dsrc", tag="dsrc")
        nc.vector.memset(dsrc[:], 0.0)
        dwarm = st_dv.tile([128, 2, QCHUNK], f32, name="st_v", tag="st_v")
        for _ in range(7):
            nc.tensor.matmul(
                dwarm[:, 0, :], lhsT=dsrc[:, 0:128], rhs=dsrc[:],
                start=True, stop=True,
            )

        state = {}

        def emit_qk_act(c, gi):
            u, qc = chunks[c]
            tl = unit_tiles[u]
            k0, kw, eng = state[c]["grps"][gi]
            q0 = qc * QCHUNK
            if eng == "s":
                st = st_sc.tile([128, 2, QCHUNK], f32, name="st_s", tag="st_s")
            else:
                st = st_dv.tile([128, 2, QCHUNK], f32, name="st_v", tag="st_v")
            for i in range(kw):
                j = k0 + i
                nc.tensor.matmul(
                    st[:, i, :],
                    lhsT=tl["k"][:, j * 128 : (j + 1) * 128],
                    rhs=tl["q"][:, q0 : q0 + QCHUNK],
                    start=True,
                    stop=True,
                )
            pt = ppool.tile([128, 2, QCHUNK], f16, name="pt", tag="pt")
            if eng == "s":
                nc.scalar.activation(
                    pt[:, 0:kw, :], st[:, 0:kw, :], EXP, bias=0.0, scale=SCALE
                )
            else:
                nc.vector.tensor_scalar(
                    out=pt[:, 0:kw, :].bitcast(i16),
                    in0=st[:, 0:kw, :],
                    scalar1=SCHRAU_A,
                    scalar2=SCHRAU_B,
                    op0=mybir.AluOpType.mult,
                    op1=mybir.AluOpType.add,
                )
            state[c]["pt"].append(pt)

        def pv_matmuls(c, t, glo, ghi, oacc):
            """PV accumulation-group matmuls for q-tile t over groups [glo, ghi)."""
            stt = state[c]
            v = stt["v"]
            first = True
            for g in range(glo, ghi):
                k0, kw, _ = stt["grps"][g]
                pt = stt["pt"][g]
                for i in range(kw):
                    nc.tensor.matmul(
                        oacc[:],
                        lhsT=pt[:, i, t * 128 : (t + 1) * 128],
                        rhs=v[:, k0 + i, 0 : D + 1],
                        start=first,
                        stop=(g == ghi - 1 and i == kw - 1),
                    )
                    first = False

        def odest(c):
            u, qc = chunks[c]
            b, h = units[u]
            return o_d[b, h, qc]

        def emit_pv(c, t):
            stt = state[c]
            oacc = o_ps.tile([128, D + 1], f32, name="oacc", tag="oacc")
            pv_matmuls(c, t, 0, len(stt["grps"]), oacc)
            rec = rpool.tile([128, 1], f32, name="rec", tag="rec")
            nc.vector.reciprocal(rec[:], oacc[:, D : D + 1])
            nc.vector.tensor_scalar_mul(stt["osb"][:, t, :], oacc[:, 0:D], rec[:])
            if t == QT - 1:
                # alternate the issuing engine so chunk stores spread across
                # DMA queues instead of serializing on one ring (gpsimd's
                # software-DGE path measured multi-us trigger->transfer
                # latency, so only sync and scalar queues are used)
                if c % 2 == 0:
                    nc.sync.dma_start(odest(c), stt["osb"][:])
                else:
                    nc.scalar.dma_start(odest(c), stt["osb"][:])
                del state[c]

        def emit_pv_part1(c, t):
            """Last chunk: PV over k-groups 0-3 (k-tiles 0-7), parked to
            SBUF; the lighter part1 keeps the final window's PE load under
            the exp window so the PE is not lagging at the last exp."""
            stt = state[c]
            oacc = o_ps.tile([128, D + 1], f32, name="oacc", tag="oacc")
            pv_matmuls(c, t, 0, 4, oacc)
            asb = apool.tile([128, D + 1], f32, name="asb", tag="asb")
            nc.vector.tensor_copy(asb[:], oacc[:])
            stt["asb"][t] = asb

        def emit_pv_part2(c, t):
            """Last chunk: PV over k-groups 4-7, combine, normalize, store."""
            stt = state[c]
            oacc = o_ps.tile([128, D + 1], f32, name="oacc", tag="oacc")
            pv_matmuls(c, t, 4, len(stt["grps"]), oacc)
            tmp = apool.tile([128, D + 1], f32, name="tmp", tag="tmp")
            nc.vector.tensor_add(tmp[:], oacc[:], stt["asb"][t][:])
            rec = rpool.tile([128, 1], f32, name="rec", tag="rec")
            nc.vector.reciprocal(rec[:], tmp[:, D : D + 1])
            nc.vector.tensor_scalar_mul(stt["osb"][:, t, :], tmp[:, 0:D], rec[:])
            # store in two contiguous halves on the two HW-DGE queues:
            # [t0,t1] as soon as t1 is normalized, [t2,t3] at the end
            if t == 1:
                nc.sync.dma_start(odest(c)[:, 0:2, :], stt["osb"][:, 0:2, :])
            elif t == QT - 1:
                nc.scalar.dma_start(odest(c)[:, 2:4, :], stt["osb"][:, 2:4, :])

        last = nchunks - 1
        for c in range(nchunks):
            u, qc = chunks[c]
            grps = GROUPS0 if c == 0 else GROUPS
            ng = len(grps)
            state[c] = {
                "pt": [],
                "asb": {},
                "grps": grps,
                "v": unit_tiles[u]["v"],
                "osb": opool.tile([128, QT, D], f16, name="osb", tag="osb"),
            }
            # prefetch the next unit's tensors two chunks ahead
            if qc == NQC - 2 and u + 1 < len(units):
                unit_tiles[u + 1] = prep(u + 1)
            for gi in range(ng):
                emit_qk_act(c, gi)
                # pv(c-1, 0) at gi=1 is safe: qk(c, g1) already waits on
                # exp(c-1, last) via the st ping-pong, the same dependency
                if c > 0 and 1 <= gi <= QT:
                    emit_pv(c - 1, gi - 1)
                if c == last and gi >= ng - 3:
                    emit_pv_part1(c, gi - (ng - 3))
            if c == last:
                emit_pv_part1(c, QT - 1)
                for t in range(QT):
                    emit_pv_part2(c, t)
                del state[c]

    nc.compile()
    return nc


def _get_program():
    if "nc" not in _CACHE:
        _CACHE["nc"] = _build_program()
    return _CACHE["nc"]


def make_core_inputs(q, k, v, key_padding_mask):
    """Shard full inputs into per-core input maps (host side).

    Layout work done here (part of sharding): head-slice, transpose Q/K to
    [d, s], cast to fp16, build ones-augmented V with masked key rows zeroed
    (exactly equivalent to -inf score masking).
    """
    q = np.asarray(q, dtype=np.float32)
    k = np.asarray(k, dtype=np.float32)
    v = np.asarray(v, dtype=np.float32)
    m = np.asarray(key_padding_mask, dtype=bool)

    # [B, S, H, D] -> [B, H, D, S] fp16
    qt = np.ascontiguousarray(q.transpose(0, 2, 3, 1).astype(np.float16))
    kt = np.ascontiguousarray(k.transpose(0, 2, 3, 1).astype(np.float16))
    # V_aug: [B, H, 128(p), KTILES(t), VW] fp16 with ones in column D,
    # masked key rows zeroed (s = 128*t + p)
    va = np.zeros((B, H, 128, KTILES, VW), dtype=np.float16)
    va[:, :, :, :, 0:D] = (
        v.reshape(B, KTILES, 128, H, D).transpose(0, 3, 2, 1, 4).astype(np.float16)
    )
    va[:, :, :, :, D] = 1.0
    if not m.all():
        # mask[b, s] with s = 128*t + p -> [B, 1, 128(p), KTILES(t), 1]
        mk = m.reshape(B, KTILES, 128).transpose(0, 2, 1)[:, None, :, :, None]
        va *= mk.astype(np.float16)

    in_maps = []
    for c in range(NCORES):
        sl = slice(c * H2, (c + 1) * H2)
        in_maps.append(
            {
                "qt": np.ascontiguousarray(qt[:, sl]),
                "kt": np.ascontiguousarray(kt[:, sl]),
                "va": np.ascontiguousarray(va[:, sl]),
            }
        )
    return in_maps


def assemble_output(results):
    """Concatenate per-core [B, H2, NQC, 128, QT, D] fp16 outputs along the
    head axis, then untile (s = qc*512 + t*128 + p) and cast to the full
    fp32 [B, S, H, D]."""
    o = np.concatenate([results[c]["o"] for c in range(NCORES)], axis=1)
    o = o.transpose(0, 2, 4, 3, 1, 5).reshape(B, S, H, D)
    return np.ascontiguousarray(o).astype(np.float32)


def kernel(q, k, v, key_padding_mask):
    from concourse.bass_utils import run_bass_kernel_spmd

    nc = _get_program()
    in_maps = make_core_inputs(q, k, v, key_padding_mask)
    res = run_bass_kernel_spmd(nc, in_maps, list(range(NCORES)))
    return assemble_output(res.results)

